# revision 18
# baseline (speedup 1.0000x reference)
"""Trainium2 Bass kernel for nn_AccentVarianceAdaptor (FastSpeech-style
variance adaptor): length-regulate encoder output by integer durations,
add quantized pitch/energy embeddings, zero the padded tail, and return
(out, lengths).

kernel(**inputs) takes the FULL unsharded numpy inputs and returns the
FULL (out [16,4096,256] f32, lengths [16] int32).  The batch dim is
sharded across 8 NeuronCores (2 batches per core); tables replicated.

Per-batch device algorithm:
  * exact cumsum of durations (f32 scan + triangular matmul prefix)
  * src[j] = searchsorted(csum, j, 'right') via a TensorEngine histogram
    (one-hot compare tiles contracted over phonemes) + inclusive scan
  * pitch/energy quantization replicating the neuron-backend searchsorted
    bit-exactly (arithmetic candidate +-1 corrected with the rounded
    bitcast-int32 key comparison its fused scan performs)
  * encoder expansion: 16 per-partition indirect DMA pair-gathers
    (2 consecutive rows per descriptor; a one-op copy_predicated fixes
    repeated rows), reading a zero-padded encoder slab for the tail
  * embeddings: one-hot matmuls against the bf16 tables accumulated in
    PSUM, added into the expanded rows; masked bins (+256) hit no one-hot
    column and contribute zero
"""

import os
import sys

import numpy as np

for _p in ("/opt/trn_rl_repo", "/root/.axon_site/_ro/trn_rl_repo"):
    if os.path.isdir(_p) and _p not in sys.path:
        sys.path.insert(0, _p)

import concourse.bass as bass
import concourse.tile as tile
from concourse import bacc, mybir
from concourse.bass_utils import run_bass_kernel_spmd
from concourse.masks import make_upper_triangular

# ---------------------------------------------------------------- constants
B, S, H = 16, 1024, 256
D_MAX = 4
T_OUT = S * D_MAX            # 4096
NB = 256                     # pitch / energy bins
NCORES = 8
BLOC = B // NCORES           # batches per core = 2
SP = S + 2                   # encoder rows per batch incl. 2 zero pad rows

F32 = mybir.dt.float32
BF16 = mybir.dt.bfloat16
I32 = mybir.dt.int32
ALU = mybir.AluOpType

_f32 = np.float32
# f32(1/255): the constant XLA folds the linspace divide into.
R255 = _f32(1.0) / _f32(255.0)
# f32(stop * (1/255)): constant-folded stop*recip multiplier in the lerp.
C2_PITCH = _f32(400.0) * R255
INV_PITCH = _f32(255.0) / _f32(350.0)   # candidate slope (any close value ok)
INV_ENERGY = _f32(255.0)

_NC_CACHE = {}


def _quantize_bins(nc, pool, v_t, maskf, vmin, vmax, inv, c2, tag):
    """[128, 32] f32 bin values replicating the device-run reference
    quantize exactly; masked positions get +256 (no one-hot column)."""
    sh = list(v_t.shape)

    def t32(name):
        return pool.tile(sh, F32, tag="qt", bufs=10, name=f"{tag}_{name}")

    v1 = t32("v1")
    nc.vector.tensor_scalar(v1[:], v_t[:], float(vmin), float(vmax), ALU.max, ALU.min)
    t = t32("t")
    nc.vector.tensor_scalar(t[:], v1[:], float(vmin), float(inv), ALU.subtract, ALU.mult)
    ci = pool.tile(sh, I32, tag="qti", bufs=4, name=f"{tag}_ci")
    nc.vector.tensor_copy(ci[:], t[:])
    cf = t32("cf")
    nc.vector.tensor_copy(cf[:], ci[:])
    g = t32("g")
    nc.vector.tensor_tensor(out=g[:], in0=cf[:], in1=t[:], op=ALU.is_lt)
    k0 = t32("k0")
    nc.vector.tensor_tensor(out=k0[:], in0=cf[:], in1=g[:], op=ALU.add)
    km1 = t32("km1")
    nc.vector.tensor_scalar(km1[:], k0[:], 1.0, None, ALU.subtract)

    def boundary(kf, nm):
        """T(k) per the XLA lerp recipe, one f32 rounding per op."""
        y = t32(nm + "_y")
        nc.vector.tensor_scalar(y[:], kf[:], float(c2), None, ALU.mult)
        if vmin == 0.0:
            return y
        u = t32(nm + "_u")
        nc.vector.tensor_scalar(u[:], kf[:], float(R255), None, ALU.mult)
        x = t32(nm + "_x")
        nc.vector.tensor_scalar(x[:], u[:], float(-vmin), float(vmin), ALU.mult, ALU.add)
        tb = t32(nm + "_T")
        nc.vector.tensor_tensor(out=tb[:], in0=x[:], in1=y[:], op=ALU.add)
        return tb

    def rkey(src_t, nm):
        """f32(bitcast_int32(x)) -- the neuron comparator's rounded key."""
        r = t32(nm + "_rk")
        nc.vector.tensor_copy(r[:], src_t[:].bitcast(I32))
        return r

    tk0 = boundary(k0, "b0")
    tkm1 = boundary(km1, "bm1")
    rv = rkey(v1, "v")
    r0 = rkey(tk0, "t0")
    rm1 = rkey(tkm1, "tm1")
    a1 = t32("a1")
    nc.vector.tensor_tensor(out=a1[:], in0=rv[:], in1=r0[:], op=ALU.is_gt)
    a2 = t32("a2")
    nc.vector.tensor_tensor(out=a2[:], in0=rv[:], in1=rm1[:], op=ALU.is_le)
    b1 = t32("b1")
    nc.vector.tensor_tensor(out=b1[:], in0=k0[:], in1=a1[:], op=ALU.add)
    b2 = t32("b2")
    nc.vector.tensor_tensor(out=b2[:], in0=b1[:], in1=a2[:], op=ALU.subtract)
    bc = t32("bc")
    nc.vector.tensor_scalar(bc[:], b2[:], 0.0, 255.0, ALU.max, ALU.min)
    bm = t32("bm")
    nc.vector.scalar_tensor_tensor(
        out=bm[:], in0=maskf[:], scalar=256.0, in1=bc[:], op0=ALU.mult, op1=ALU.add
    )
    return bm


def build_nc():
    nc = bacc.Bacc("TRN2", target_bir_lowering=False, debug=False)

    enc = nc.dram_tensor("enc", [BLOC * SP, H], F32, kind="ExternalInput").ap()
    pit = nc.dram_tensor("pit", [BLOC, T_OUT], F32, kind="ExternalInput").ap()
    ene = nc.dram_tensor("ene", [BLOC, T_OUT], F32, kind="ExternalInput").ap()
    dur = nc.dram_tensor("dur", [BLOC, S], F32, kind="ExternalInput").ap()
    ptab = nc.dram_tensor("ptab", [NB, H], F32, kind="ExternalInput").ap()
    etab = nc.dram_tensor("etab", [NB, H], F32, kind="ExternalInput").ap()
    out = nc.dram_tensor("out", [BLOC, T_OUT, H], F32, kind="ExternalOutput").ap()
    lens = nc.dram_tensor("lens", [BLOC, 1], I32, kind="ExternalOutput").ap()
    # scratch for the bins broadcast roundtrip: [batch, table, 4096]
    sbins = nc.dram_tensor("sbins", [BLOC, 2, T_OUT], F32, kind="Internal").ap()

    from contextlib import ExitStack

    with tile.TileContext(nc) as tc, ExitStack() as ctx:
        const = ctx.enter_context(tc.tile_pool(name="const", bufs=1))
        small = ctx.enter_context(tc.tile_pool(name="small", bufs=2))
        big = ctx.enter_context(tc.tile_pool(name="big", bufs=2))
        psum = ctx.enter_context(tc.tile_pool(name="psum", bufs=1, space="PSUM"))

        # ---- one-time constants
        iota128_i = const.tile([128, 128], I32)
        nc.gpsimd.iota(iota128_i[:], pattern=[[1, 128]], base=0, channel_multiplier=0)
        iota128 = const.tile([128, 128], F32)
        nc.vector.tensor_copy(iota128[:], iota128_i[:])
        iota32_i = const.tile([128, 32], I32)
        nc.gpsimd.iota(iota32_i[:], pattern=[[1, 32]], base=0, channel_multiplier=0)
        iota32 = const.tile([128, 32], F32)
        nc.vector.tensor_copy(iota32[:], iota32_i[:])
        ustrict = const.tile([128, 128], F32)
        make_upper_triangular(nc, ustrict[:], val=1.0, diag=False)
        ones128 = const.tile([128, 128], F32)
        nc.vector.memset(ones128[:], 1.0)
        zeros8 = const.tile([128, 8], F32)
        nc.vector.memset(zeros8[:], 0.0)
        zeros32 = const.tile([128, 32], F32)
        nc.vector.memset(zeros32[:], 0.0)
        # per-partition chunk-base columns for the one-hot compares
        iotac0_i = const.tile([128, 1], I32)
        nc.gpsimd.iota(iotac0_i[:], pattern=[[0, 1]], base=0, channel_multiplier=1)
        iotac0 = const.tile([128, 1], F32)
        nc.vector.tensor_copy(iotac0[:], iotac0_i[:])
        iotac1 = const.tile([128, 1], F32)
        nc.vector.tensor_scalar(iotac1[:], iotac0[:], 128.0, None, ALU.add)
        # embedding tables as bf16 chunk tiles (matmul rhs)
        tabs = []
        for nm, tab in (("pt", ptab), ("et", etab)):
            for c in range(2):
                tf = const.tile([128, H], F32, name=f"{nm}{c}f")
                nc.sync.dma_start(out=tf[:], in_=tab[c * 128 : (c + 1) * 128, :])
                tabs.append(tf)
        pt0, pt1, et0, et1 = tabs

        for b in range(BLOC):
            # ---- loads
            dur_t = small.tile([128, 8], F32, tag="dur")
            nc.sync.dma_start(out=dur_t[:], in_=dur[b].rearrange("(p f) -> p f", p=128))
            pit_t = small.tile([128, 32], F32, tag="pit")
            nc.sync.dma_start(out=pit_t[:], in_=pit[b].rearrange("(p f) -> p f", p=128))
            ene_t = small.tile([128, 32], F32, tag="ene")
            nc.sync.dma_start(out=ene_t[:], in_=ene[b].rearrange("(p f) -> p f", p=128))

            # ---- csum over durations (i = 8p + f), exact in f32
            sd = small.tile([128, 8], F32, tag="sd")
            nc.vector.tensor_tensor_scan(
                out=sd[:], data0=dur_t[:], data1=zeros8[:], initial=0.0,
                op0=ALU.add, op1=ALU.add,
            )
            offs1 = psum.tile([128, 1], F32, tag="sps", bufs=2, name="offs1")
            nc.tensor.matmul(offs1[:], lhsT=ustrict[:], rhs=sd[:, 7:8], start=True, stop=True)
            csum_f = small.tile([128, 8], F32, tag="csumf")
            nc.vector.tensor_scalar(csum_f[:], sd[:], offs1[:, 0:1], None, ALU.add)

            # ---- lengths
            tot_ps = psum.tile([128, 1], F32, tag="sps", bufs=2, name="tot_ps")
            nc.tensor.matmul(tot_ps[:], lhsT=ones128[:], rhs=sd[:, 7:8], start=True, stop=True)
            len_i = small.tile([1, 1], I32, tag="leni")
            nc.vector.tensor_copy(len_i[:], tot_ps[0:1, 0:1])
            nc.sync.dma_start(out=lens[b : b + 1, :], in_=len_i[:])

            # ---- histogram of csum on [128, 32] (j = 32p + f)
            csum_i = small.tile([128, 8], I32, tag="csumi")
            nc.vector.tensor_copy(csum_i[:], csum_f[:])
            cd_i = small.tile([128, 8], I32, tag="cdi")
            nc.vector.tensor_scalar(cd_i[:], csum_i[:], 5, None, ALU.logical_shift_right)
            cm_i = small.tile([128, 8], I32, tag="cmi")
            nc.vector.tensor_scalar(cm_i[:], csum_i[:], 31, None, ALU.bitwise_and)
            cd = small.tile([128, 8], F32, tag="cd")
            nc.vector.tensor_copy(cd[:], cd_i[:])
            cm = small.tile([128, 8], F32, tag="cm")
            nc.vector.tensor_copy(cm[:], cm_i[:])

            h_ps = psum.tile([128, 32], F32, tag="hps", bufs=2, name="h_ps")
            for f in range(8):
                hi = small.tile([128, 128], F32, tag="hi", bufs=3, name="hi")
                nc.vector.tensor_scalar(hi[:], iota128[:], cd[:, f : f + 1], None, ALU.is_equal)
                lo = small.tile([128, 32], F32, tag="lo", bufs=3, name="lo")
                nc.vector.tensor_scalar(lo[:], iota32[:], cm[:, f : f + 1], None, ALU.is_equal)
                nc.tensor.matmul(h_ps[:], lhsT=hi[:], rhs=lo[:], start=(f == 0), stop=(f == 7))

            # ---- src[j] = inclusive-cumsum(h), j = 32p + f
            h_sb = small.tile([128, 32], F32, tag="hsb")
            nc.vector.tensor_copy(h_sb[:], h_ps[:])
            s1 = small.tile([128, 32], F32, tag="s1")
            nc.vector.tensor_tensor_scan(
                out=s1[:], data0=h_sb[:], data1=zeros32[:], initial=0.0,
                op0=ALU.add, op1=ALU.add,
            )
            offs2 = psum.tile([128, 1], F32, tag="sps", bufs=2, name="offs2")
            nc.tensor.matmul(offs2[:], lhsT=ustrict[:], rhs=s1[:, 31:32], start=True, stop=True)
            src_f = small.tile([128, 32], F32, tag="srcf")
            nc.vector.tensor_scalar(src_f[:], s1[:], offs2[:, 0:1], None, ALU.add)
            maskf = small.tile([128, 32], F32, tag="maskf")
            nc.vector.tensor_scalar(maskf[:], src_f[:], float(S), None, ALU.is_equal)

            # ---- quantized bins -> DRAM roundtrip -> broadcast rows
            pbin = _quantize_bins(nc, small, pit_t, maskf, 50.0, 400.0, INV_PITCH, C2_PITCH, "p")
            ebin = _quantize_bins(nc, small, ene_t, maskf, 0.0, 1.0, INV_ENERGY, R255, "e")
            nc.sync.dma_start(out=sbins[b, 0].rearrange("(p f) -> p f", p=128), in_=pbin[:])
            nc.sync.dma_start(out=sbins[b, 1].rearrange("(p f) -> p f", p=128), in_=ebin[:])
            pbb = big.tile([128, T_OUT], F32, tag="bb", bufs=2, name="pbb")
            nc.sync.dma_start(out=pbb[:], in_=sbins[b, 0].rearrange("(a f) -> a f", a=1).to_broadcast([128, T_OUT]))
            ebb = big.tile([128, T_OUT], F32, tag="bb", bufs=2, name="ebb")
            nc.sync.dma_start(out=ebb[:], in_=sbins[b, 1].rearrange("(a f) -> a f", a=1).to_broadcast([128, T_OUT]))

            # ---- one-hot tiles [c-chunk 128, j 4096]
            ohp0 = big.tile([128, T_OUT], F32, tag="ohp0", bufs=1, name="ohp0")
            nc.vector.tensor_scalar(ohp0[:], pbb[:], iotac0[:, 0:1], None, ALU.is_equal)
            ohp1 = big.tile([128, T_OUT], F32, tag="ohp1", bufs=1, name="ohp1")
            nc.vector.tensor_scalar(ohp1[:], pbb[:], iotac1[:, 0:1], None, ALU.is_equal)
            # energy one-hots on the scalar engine: relu(1 - |bb - c|)
            et0a = big.tile([128, T_OUT], F32, tag="bb", bufs=2, name="et0a")
            negc0 = small.tile([128, 1], F32, tag="negc0")
            nc.vector.tensor_scalar(negc0[:], iotac0[:], -1.0, None, ALU.mult)
            negc1 = small.tile([128, 1], F32, tag="negc1")
            nc.vector.tensor_scalar(negc1[:], iotac1[:], -1.0, None, ALU.mult)
            nc.scalar.activation(et0a[:], ebb[:], mybir.ActivationFunctionType.Abs,
                                 bias=negc0[:, 0:1], scale=1.0)
            ohe0 = big.tile([128, T_OUT], F32, tag="ohe0", bufs=1, name="ohe0")
            nc.scalar.activation(ohe0[:], et0a[:], mybir.ActivationFunctionType.Relu,
                                 bias=1.0, scale=-1.0)
            et1a = big.tile([128, T_OUT], F32, tag="bb", bufs=2, name="et1a")
            nc.scalar.activation(et1a[:], ebb[:], mybir.ActivationFunctionType.Abs,
                                 bias=negc1[:, 0:1], scale=1.0)
            ohe1 = big.tile([128, T_OUT], F32, tag="ohe1", bufs=1, name="ohe1")
            nc.scalar.activation(ohe1[:], et1a[:], mybir.ActivationFunctionType.Relu,
                                 bias=1.0, scale=-1.0)

            # ---- encoder expansion: 16 pair-gathers + repeat fix
            base_i = small.tile([128, 16], I32, tag="basei")
            nc.vector.tensor_copy(base_i[:], src_f[:, 0:32:2])
            rep = small.tile([128, 16], I32, tag="rep")
            nc.vector.tensor_tensor(out=rep[:], in0=src_f[:, 1:32:2], in1=src_f[:, 0:32:2],
                                    op=ALU.is_equal)
            acc = big.tile([128, 32 * H], F32, tag="acc", bufs=2, name="acc")
            for gidx in range(16):
                nc.gpsimd.indirect_dma_start(
                    out=acc[:, gidx * 2 * H : (gidx * 2 + 2) * H],
                    out_offset=None,
                    in_=enc,
                    in_offset=bass.IndirectOffsetOnAxis(ap=base_i[:, gidx : gidx + 1], axis=0),
                    element_offset=b * SP * H,
                )
            acc3 = acc[:].rearrange("p (f h) -> p f h", h=H)
            nc.vector.copy_predicated(
                out=acc3[:, 1:32:2, :],
                mask=rep[:].to_broadcast([128, 16, H]),
                data=acc3[:, 0:32:2, :],
            )

            # ---- embedding matmuls + add into acc, per 128-row j block
            for f in range(0, 32, 2):
                tps = psum.tile([128, 2 * H], F32, tag="tps", bufs=3, name="tps")
                for half in range(2):
                    sl = slice(f + half, T_OUT, 32)
                    o = tps[:, half * H : (half + 1) * H]
                    nc.tensor.matmul(o, lhsT=ohp0[:, sl], rhs=pt0[:], start=True, stop=False)
                    nc.tensor.matmul(o, lhsT=ohp1[:, sl], rhs=pt1[:], start=False, stop=False)
                    nc.tensor.matmul(o, lhsT=ohe0[:, sl], rhs=et0[:], start=False, stop=False)
                    nc.tensor.matmul(o, lhsT=ohe1[:, sl], rhs=et1[:], start=False, stop=True)
                nc.vector.tensor_tensor(
                    out=acc[:, f * H : (f + 2) * H], in0=acc[:, f * H : (f + 2) * H],
                    in1=tps[:], op=ALU.add,
                )

            # ---- store: partition p holds output rows [32p, 32p+32)
            nc.sync.dma_start(
                out=out[b].rearrange("(p x) h -> p (x h)", p=128), in_=acc[:]
            )

    nc.compile()
    return nc


def get_nc():
    if "nc" not in _NC_CACHE:
        _NC_CACHE["nc"] = build_nc()
    return _NC_CACHE["nc"]


def make_in_maps(encoder_output, pitch_target, energy_target, duration_target,
                 pitch_table, energy_table):
    enc = np.ascontiguousarray(encoder_output, dtype=np.float32)
    pit = np.ascontiguousarray(pitch_target, dtype=np.float32)
    ene = np.ascontiguousarray(energy_target, dtype=np.float32)
    dur = np.ascontiguousarray(duration_target, dtype=np.float32)
    ptab = np.ascontiguousarray(pitch_table, dtype=np.float32)
    etab = np.ascontiguousarray(energy_table, dtype=np.float32)
    # two zero pad rows per batch: masked frames gather rows S / S+1
    encp = np.concatenate([enc, np.zeros((B, 2, H), np.float32)], axis=1)
    in_maps = []
    for c in range(NCORES):
        sl = slice(c * BLOC, (c + 1) * BLOC)
        in_maps.append({
            "enc": encp[sl].reshape(BLOC * SP, H),
            "pit": pit[sl],
            "ene": ene[sl],
            "dur": dur[sl],
            "ptab": ptab,
            "etab": etab,
        })
    return in_maps


def kernel(encoder_output, pitch_target, energy_target, duration_target,
           pitch_table, energy_table, _trace=False):
    nc = get_nc()
    in_maps = make_in_maps(encoder_output, pitch_target, energy_target,
                           duration_target, pitch_table, energy_table)
    res = run_bass_kernel_spmd(nc, in_maps, core_ids=list(range(NCORES)),
                               trace=_trace)
    outs = res.results
    out = np.concatenate([outs[c]["out"] for c in range(NCORES)], axis=0)
    lens = np.concatenate([outs[c]["lens"].reshape(BLOC) for c in range(NCORES)])
    kernel.last_results = res
    return out, lens.astype(np.int32)
